# revision 13
# baseline (speedup 1.0000x reference)
"""Trainium2 Bass kernel for nn_Attention_56470230008033.

Multi-head self-attention (B=2, N=2048, C=1024, H=16 heads, D=64),
k = v = q, full qkv projection + output projection.

Sharding over 8 NeuronCores: data parallel on batch (2) x tensor
parallel on heads (4 head-groups of 4 heads).

v2: single fused pipeline built around the ScalarE exp roofline
(~147us of ACT work per core).  The softmax exp stream is kept
saturated from ~11us on; every other engine's work (x load/transpose,
qkv projections, PV, output projection) is interleaved underneath it:
  - attention processed per (query-quarter 512, head-pair), window =
    2 key-tiles; logits matmuls run as row-tiled concurrent pairs
    (tile_position (0,0)/(64,0), K=64 each)
  - logits land in bf16 PSUM (1 bank per head-window) so the whole
    working set fits the 8 PSUM banks:
      4 x bp (bf16 [128,1024]) + 2 x cp ([65,512] f32) + 2 x scratch
  - denominators ride the PV matmul as a ones-column (M=65)
  - output projection packs head pairs (K=128) and shares the scratch
    PSUM pool; y is written bf16 (host sums partials in f32)
"""

import sys

for _p in ("/opt/trn_rl_repo", "/opt/pypackages"):
    if _p not in sys.path:
        sys.path.append(_p)

import numpy as np

B, N, C, H = 2, 2048, 1024, 16
D = C // H            # 64 head dim
NCORES = 8
HPC = 4               # heads per core
F = HPC * D           # 256 features per core
NT = N // 128         # 16 token tiles
CT = C // 128         # 8 contraction tiles

PAIRED = True         # row-tiled concurrent logits pairs

_CACHE = {}


def _build():
    from concourse import bacc, bass, mybir, tile, masks

    F32 = mybir.dt.float32
    BF16 = mybir.dt.bfloat16
    AF = mybir.ActivationFunctionType

    nc = bacc.Bacc(
        "TRN2",
        target_bir_lowering=False,
        debug=False,
        enable_asserts=False,
        num_devices=NCORES,
    )
    x_d = nc.dram_tensor("x", [N, C], F32, kind="ExternalInput")
    wqk_d = nc.dram_tensor("wqk", [C, 2 * F], F32, kind="ExternalInput")
    wv_d = nc.dram_tensor("wv", [C, F], F32, kind="ExternalInput")
    wp_d = nc.dram_tensor("wp", [F, C], F32, kind="ExternalInput")
    bqk_d = nc.dram_tensor("bqk", [2 * F, 1], F32, kind="ExternalInput")
    bv_d = nc.dram_tensor("bv", [1, F], F32, kind="ExternalInput")
    y_d = nc.dram_tensor("y", [N, C], BF16, kind="ExternalOutput")

    scale = float(D) ** -0.5

    with tile.TileContext(nc) as tc:
        from contextlib import ExitStack

        with ExitStack() as ctx:
            const = ctx.enter_context(tc.tile_pool(name="const", bufs=1))
            persist = ctx.enter_context(tc.tile_pool(name="persist", bufs=1))

            ident = const.tile([128, 128], BF16, name="ident", tag="ident")
            masks.make_identity(nc, ident[:])

            # x^T, laid out token-tile-major: [p, t, c, tok]
            xT4 = persist.tile([128, NT, CT, 128], BF16, name="xT4", tag="xT4")
            # qkT[0..1] = Q^T head-pairs, qkT[2..3] = K^T head-pairs
            qkT = [persist.tile([128, N], BF16, name=f"qkT{f}", tag=f"qkT{f}")
                   for f in range(4)]
            # V with ones column per head: cols [65h .. 65h+64]
            vaug = [persist.tile([128, 65 * HPC], BF16, name=f"vaug{t}", tag=f"vaug{t}")
                    for t in range(NT)]
            # O^T stacked per head pair (rows 0-63 head 2p, 64-127 head 2p+1)
            oT2 = [persist.tile([128, N], BF16, name=f"oT2{p}", tag=f"oT2{p}")
                   for p in range(2)]
            wqk = [persist.tile([128, 2 * F], BF16, name=f"wqk{c}", tag=f"wqk{c}")
                   for c in range(CT)]
            wv = [persist.tile([128, F], BF16, name=f"wv{c}", tag=f"wv{c}")
                  for c in range(CT)]
            wp2 = [persist.tile([128, C], BF16, name=f"wp2{p}", tag=f"wp2{p}")
                   for p in range(2)]
            bqk_sb = [const.tile([128, 1], F32, name=f"bqk{f}", tag=f"bqk{f}")
                      for f in range(4)]
            bvb = const.tile([128, F], F32, name="bvb", tag="bvb")

            # preload the exp table set immediately (one-time ~2.7us)
            scr = const.tile([1, 16], F32, name="scr", tag="scr")
            nc.scalar.activation(scr[:], ident[0:1, 0:16], AF.Exp)

            # ---------------- DMA issue (front-loaded) ----------------
            xload = ctx.enter_context(tc.tile_pool(name="xload", bufs=4))
            wstage = ctx.enter_context(tc.tile_pool(name="wstage", bufs=2))
            xbp = ctx.enter_context(tc.tile_pool(name="xbp", bufs=2))
            ptp = ctx.enter_context(tc.tile_pool(name="ptp", bufs=20))
            snorm = ctx.enter_context(tc.tile_pool(name="snorm", bufs=2))
            ysb = ctx.enter_context(tc.tile_pool(name="ysb", bufs=2))

            bpp = ctx.enter_context(
                tc.tile_pool(name="bpp", bufs=2, space=bass.MemorySpace.PSUM))
            cpp = ctx.enter_context(
                tc.tile_pool(name="cpp", bufs=2, space=bass.MemorySpace.PSUM))
            aux = ctx.enter_context(
                tc.tile_pool(name="aux", bufs=2, space=bass.MemorySpace.PSUM))

            # x rows (tp*256 + i*128 + p) -> xs[p, i*1024 + c]
            x_view = x_d.ap().rearrange("(tp i p) c -> tp p i c", tp=NT // 2, i=2)
            xs_tiles = []
            for tp in range(NT // 2):
                xs = xload.tile([128, 2048], F32, name="xs", tag="xs")
                xsv = xs.rearrange("p (i c) -> p i c", i=2)
                if tp < 2:
                    for i in range(2):
                        nc.sync.dma_start(xsv[:, i], x_view[tp][:, i])
                else:
                    nc.sync.dma_start(xsv, x_view[tp])
                xs_tiles.append(xs)
                if tp == 0:
                    # weights on the gpsimd ring, immediately
                    wqk_stage = []
                    for c in range(CT):
                        s = wstage.tile([128, 2 * F], F32, name="wqks", tag="wqks")
                        nc.gpsimd.dma_start(s[:], wqk_d.ap()[c * 128:(c + 1) * 128, :])
                        wqk_stage.append(s)
                    for f in range(4):
                        nc.gpsimd.dma_start(bqk_sb[f][:], bqk_d.ap()[f * 128:(f + 1) * 128, :])

            # ---------------- helper emitters ----------------
            def cast_tp(tp):
                # f32 staging -> bf16 token-tile pair
                xb = xbp.tile([128, 2048], BF16, name="xb", tag="xb")
                nc.vector.tensor_copy(xb[:], xs_tiles[tp][:])
                return xb

            def transpose_quad(xb, t, cq):
                # 4 c-tiles of token-tile t -> xT4[:, t, 4cq:4cq+4, :]
                tq = aux.tile([128, 512], BF16, name="tq", tag="aux")
                i = t % 2
                for j in range(4):
                    c = 4 * cq + j
                    nc.tensor.transpose(
                        tq[:, j * 128:(j + 1) * 128],
                        xb[:, i * 1024 + c * 128:i * 1024 + (c + 1) * 128],
                        ident[:])
                nc.vector.tensor_copy(
                    xT4[:, t, 4 * cq:4 * cq + 4, :],
                    tq.rearrange("p (c n) -> p c n", c=4))

            def qk_unit(f, ch):
                # qkT[f][:, ch*512:(ch+1)*512] via 8 c-tile matmuls
                qp = aux.tile([128, 512], F32, name="qp", tag="aux")
                t0 = ch * 4
                for c in range(CT):
                    nc.tensor.matmul(
                        qp[:],
                        wqk[c][:, f * 128:(f + 1) * 128],
                        xT4[:, t0:t0 + 4, c, :],
                        start=(c == 0), stop=(c == CT - 1))
                nc.vector.tensor_scalar_add(
                    qkT[f][:, ch * 512:(ch + 1) * 512], qp[:], bqk_sb[f][:])

            def v_unit(t):
                vp = aux.tile([128, F], F32, name="vp", tag="aux")
                for c in range(CT):
                    nc.tensor.matmul(
                        vp[:], xT4[:, t, c, :], wv[c][:],
                        start=(c == 0), stop=(c == CT - 1))
                for h in range(HPC):
                    nc.vector.memset(vaug[t][:, 65 * h + 64:65 * h + 65], 1.0)
                vv = vaug[t].rearrange("p (h d) -> p h d", h=HPC)
                nc.vector.tensor_add(
                    vv[:, :, 0:D],
                    vp.rearrange("p (h d) -> p h d", h=HPC),
                    bvb.rearrange("p (h d) -> p h d", h=HPC))

            def yp_unit(t):
                # output projection for token tile t (both head pairs)
                for ch in range(2):
                    yp = aux.tile([128, 512], F32, name="yp", tag="aux")
                    for p in range(2):
                        nc.tensor.matmul(
                            yp[:],
                            oT2[p][:, t * 128:(t + 1) * 128],
                            wp2[p][:, ch * 512:(ch + 1) * 512],
                            start=(p == 0), stop=(p == 1))
                    ys = ysb.tile([128, 512], BF16, name="ys", tag="ys")
                    nc.vector.tensor_copy(ys[:], yp[:])
                    nc.sync.dma_start(
                        y_d.ap()[t * 128:(t + 1) * 128, ch * 512:(ch + 1) * 512],
                        ys[:])

            # ---------------- filler queue ----------------
            # each entry emits a bounded chunk of PE work + its DVE evac
            fillers = []

            def add_cast_and_transpose(tp):
                def emit():
                    xb = cast_tp(tp)
                    for i in range(2):
                        t = 2 * tp + i
                        for cq in range(2):
                            transpose_quad(xb, t, cq)
                fillers.append(emit)

            def defer(fn, *a):
                fillers.append(lambda: fn(*a))

            # prefix: weights for wv/wp/bv after wqk (still early)
            def load_wv_wp():
                wv_stage = []
                for c in range(CT):
                    s2 = wstage.tile([128, F], F32, name="wvs", tag="wvs")
                    nc.gpsimd.dma_start(s2[:], wv_d.ap()[c * 128:(c + 1) * 128, :])
                    wv_stage.append(s2)
                wp_stage = []
                for p in range(2):
                    s3 = wstage.tile([128, C], F32, name="wps", tag="wps")
                    nc.gpsimd.dma_start(s3[:], wp_d.ap()[p * 128:(p + 1) * 128, :])
                    wp_stage.append(s3)
                bv1 = const.tile([1, F], F32, name="bv1", tag="bv1")
                nc.gpsimd.dma_start(bv1[:], bv_d.ap()[:])
                nc.gpsimd.partition_broadcast(bvb[:], bv1[:])
                return wv_stage, wp_stage

            # ---------------- prefix ----------------
            # cast+transpose t0..t3, first Q/K chunks, start attention
            wv_stage, wp_stage = load_wv_wp()
            for c in range(CT):
                nc.vector.tensor_copy(wqk[c][:], wqk_stage[c][:])
            for tp in range(2):
                xb = cast_tp(tp)
                for i in range(2):
                    t = 2 * tp + i
                    for cq in range(2):
                        transpose_quad(xb, t, cq)
            qk_unit(2, 0)   # K^T heads 0,1 keys 0-511
            qk_unit(0, 0)   # Q^T heads 0,1 queries 0-511

            # weight casts + remaining transposes + v_proj as fillers,
            # ordered so dependencies land comfortably early
            def cast_wv():
                for c in range(CT):
                    nc.vector.tensor_copy(wv[c][:], wv_stage[c][:])

            def cast_wp():
                for p in range(2):
                    nc.vector.tensor_copy(wp2[p][:], wp_stage[p][:])

            defer(cast_wv)
            # exp-feeding chain (casts/transposes/K-Q projections) is
            # prioritized; the PV-feeding chain (v_proj) can lag -- the
            # deep pt ring buffer lets PV drift behind the exp stream.
            add_cast_and_transpose(2)
            add_cast_and_transpose(3)
            defer(qk_unit, 2, 1)    # K(0,1) keys 512-1023, by window 2
            add_cast_and_transpose(4)
            add_cast_and_transpose(5)
            defer(qk_unit, 2, 2)    # by window 4
            add_cast_and_transpose(6)
            add_cast_and_transpose(7)
            defer(qk_unit, 2, 3)    # by window 6
            defer(v_unit, 0)
            defer(qk_unit, 3, 0)    # K(2,3) keys 0-511 (pair 1, window 8)
            defer(v_unit, 1)
            defer(qk_unit, 1, 0)    # Q(2,3) queries 0-511 (pair 1)
            defer(v_unit, 2)
            defer(v_unit, 3)
            defer(qk_unit, 3, 1)
            defer(v_unit, 4)
            defer(v_unit, 5)
            defer(qk_unit, 3, 2)
            defer(v_unit, 6)
            defer(v_unit, 7)
            defer(qk_unit, 3, 3)
            for t in range(8, 16):
                defer(v_unit, t)
            defer(qk_unit, 0, 1)    # Q chunks for later quarters
            defer(qk_unit, 1, 1)
            defer(cast_wp)
            for q in range(2, 4):
                defer(qk_unit, 0, q)
                defer(qk_unit, 1, q)

            # ---------------- fused attention ----------------
            FPW = 1   # fillers per exp unit

            def emit_fillers(n):
                for _ in range(n):
                    if fillers:
                        fillers.pop(0)()

            def pv_pair(ptent, cp_h, cp_hp, pr):
                pt, mt = ptent
                h0 = 65 * (2 * pr)
                h1 = 65 * (2 * pr + 1)
                st, sp = (mt == 0), (mt == NT - 1)
                nc.tensor.matmul(
                    cp_h[:], vaug[mt][:, h0:h0 + 65], pt[:, 0],
                    start=st, stop=sp)
                nc.tensor.matmul(
                    cp_hp[:], vaug[mt][:, h1:h1 + 65], pt[:, 1],
                    start=st, stop=sp)

            for quarter in range(4):
                qs = quarter * 512
                for pr in range(2):
                    qt = qkT[pr]
                    kt = qkT[2 + pr]
                    cp_h = cpp.tile([65, 512], F32, name="cph", tag="cp")
                    cp_hp = cpp.tile([65, 512], F32, name="cphp", tag="cp")
                    pts = []   # pending (pt, mt) for PV, lagging one unit
                    for mt in range(NT):
                        # bp[:, 0] (head 2pr) -> bank 2i, bp[:, 1] -> 2i+1:
                        # the row-tiled pair writes different banks
                        bp = bpp.tile([128, 2, 512], F32, name="bp", tag="bp")
                        tp0 = dict(tile_position=(0, 0)) if PAIRED else {}
                        tp1 = dict(tile_position=(64, 0)) if PAIRED else {}
                        nc.tensor.matmul(
                            bp[:, 0], kt[0:64, mt * 128:(mt + 1) * 128],
                            qt[0:64, qs:qs + 512],
                            start=True, stop=True, **tp0)
                        nc.tensor.matmul(
                            bp[:, 1], kt[64:128, mt * 128:(mt + 1) * 128],
                            qt[64:128, qs:qs + 512],
                            start=True, stop=True, **tp1)
                        pt = ptp.tile([128, 2, 512], BF16, name="pt", tag="pt")
                        nc.scalar.activation(pt[:], bp[:], AF.Exp, scale=scale)
                        pts.append((pt, mt))
                        # PV lags one unit so exp never waits on PE
                        if len(pts) > 1:
                            pv_pair(pts.pop(0), cp_h, cp_hp, pr)
                        emit_fillers(FPW)
                    pv_pair(pts.pop(0), cp_h, cp_hp, pr)
                    # normalize -> oT2
                    for rb, cp in ((0, cp_h), (64, cp_hp)):
                        s0 = snorm.tile([1, 512], F32, name="s0", tag="s0")
                        nc.vector.tensor_copy(s0[:], cp[64:65, :])
                        sr = snorm.tile([1, 512], F32, name="sr", tag="sr")
                        nc.vector.reciprocal_approx_fast(sr[:], s0[:])
                        sb = snorm.tile([64, 512], F32, name="sb", tag="sb")
                        nc.gpsimd.partition_broadcast(sb[:], sr[:])
                        nc.vector.tensor_mul(
                            oT2[pr][rb:rb + 64, qs:qs + 512], cp[0:64, :], sb[:])
                # output projection for completed quarters (lag one)
                if quarter >= 1:
                    for t in range(4 * (quarter - 1), 4 * quarter):
                        defer(yp_unit, t)

            # tail: drain remaining fillers + last quarter's projection
            while fillers:
                fillers.pop(0)()
            for t in range(12, 16):
                yp_unit(t)

    nc.compile()
    return nc


def _get_nc():
    if "nc" not in _CACHE:
        _CACHE["nc"] = _build()
    return _CACHE["nc"]


def _in_maps(q, W_qkv, b_qkv, W_proj):
    maps = []
    for core in range(NCORES):
        b, g = divmod(core, HPC)
        cols = slice(g * F, (g + 1) * F)
        maps.append({
            "x": q[b],
            "wqk": np.ascontiguousarray(
                np.concatenate([W_qkv[:, cols], W_qkv[:, C:][:, cols]], axis=1)),
            "wv": np.ascontiguousarray(W_qkv[:, 2 * C:][:, cols]),
            "wp": np.ascontiguousarray(W_proj[cols, :]),
            "bqk": np.ascontiguousarray(
                np.concatenate([b_qkv[cols], b_qkv[C:][cols]]).reshape(2 * F, 1)),
            "bv": np.ascontiguousarray(b_qkv[2 * C:][cols].reshape(1, F)),
        })
    return maps


def kernel(q, W_qkv, b_qkv, W_proj, b_proj):
    from concourse.bass_utils import run_bass_kernel_spmd

    q = np.ascontiguousarray(np.asarray(q, dtype=np.float32))
    W_qkv = np.ascontiguousarray(np.asarray(W_qkv, dtype=np.float32))
    b_qkv = np.ascontiguousarray(np.asarray(b_qkv, dtype=np.float32))
    W_proj = np.ascontiguousarray(np.asarray(W_proj, dtype=np.float32))
    b_proj = np.ascontiguousarray(np.asarray(b_proj, dtype=np.float32))

    nc = _get_nc()
    res = run_bass_kernel_spmd(nc, _in_maps(q, W_qkv, b_qkv, W_proj),
                               core_ids=list(range(NCORES)))

    out = np.zeros((B, N, C), dtype=np.float32)
    for core in range(NCORES):
        out[core // HPC] += np.asarray(res.results[core]["y"], dtype=np.float32)
    out += b_proj
    return out


# revision 16
# speedup vs baseline: 1.2058x; 1.2058x over previous
"""Trainium2 Bass kernel for nn_Attention_56470230008033.

Multi-head self-attention (B=2, N=2048, C=1024, H=16 heads, D=64),
k = v = q, full qkv projection + output projection.

Sharding over 8 NeuronCores: data parallel on batch (2) x tensor
parallel on heads (4 head-groups of 4 heads).

v3: single fused pipeline.  The PE is the bottleneck engine
(~200us: logits + PV are PSUM-drain-bound at 2x216ns per key-tile;
projections/transposes on top); the ScalarE exp stream (~171us) runs
underneath it.  Structure:
  - attention processed per (query-quarter 512, head-pair); unit =
    one key-tile: bp [128, 2, 512] f32 spans 2 PSUM banks (head h ->
    bank 2i, h' -> 2i+1), one exp [128,1024] covers both heads
  - PSUM: 2x bp (4 banks) + 2 cp [65,512] + 2 shared scratch = 8
  - denominators ride the PV matmul as a ones-column (M=65)
  - x load / transpose / qkv projections / output projection are
    emitted as fine-grained "fillers" inside the attention loop in
    deadline order; PV lags 3 units; normalize is deferred into the
    next pair so nothing blocks the exp stream
  - weights: wqk on the scalar HWDGE ring (fast path, needed first),
    x on sync ring, wv/wp/biases on the gpsimd ring
  - y written bf16 (host sums partials in f32)
"""

import sys

for _p in ("/opt/trn_rl_repo", "/opt/pypackages"):
    if _p not in sys.path:
        sys.path.append(_p)

import numpy as np

B, N, C, H = 2, 2048, 1024, 16
D = C // H            # 64 head dim
NCORES = 8
HPC = 4               # heads per core
F = HPC * D           # 256 features per core
NT = N // 128         # 16 token tiles
CT = C // 128         # 8 contraction tiles

PAIRED = True         # row-tiled logits pairs (tile_position)
PVLAG = 3             # PV units of lag behind exp

_CACHE = {}


def _build():
    from concourse import bacc, bass, mybir, tile, masks

    F32 = mybir.dt.float32
    BF16 = mybir.dt.bfloat16
    AF = mybir.ActivationFunctionType

    nc = bacc.Bacc(
        "TRN2",
        target_bir_lowering=False,
        debug=False,
        enable_asserts=False,
        num_devices=NCORES,
    )
    x_d = nc.dram_tensor("x", [N, C], F32, kind="ExternalInput")
    wqk_d = nc.dram_tensor("wqk", [C, 2 * F], F32, kind="ExternalInput")
    wv_d = nc.dram_tensor("wv", [C, F], F32, kind="ExternalInput")
    wp_d = nc.dram_tensor("wp", [F, C], F32, kind="ExternalInput")
    bqk_d = nc.dram_tensor("bqk", [2 * F, 1], F32, kind="ExternalInput")
    bv_d = nc.dram_tensor("bv", [1, F], F32, kind="ExternalInput")
    y_d = nc.dram_tensor("y", [N, C], BF16, kind="ExternalOutput")

    scale = float(D) ** -0.5

    with tile.TileContext(nc) as tc:
        from contextlib import ExitStack

        with ExitStack() as ctx:
            const = ctx.enter_context(tc.tile_pool(name="const", bufs=1))
            persist = ctx.enter_context(tc.tile_pool(name="persist", bufs=1))

            ident = const.tile([128, 128], BF16, name="ident", tag="ident")
            masks.make_identity(nc, ident[:])

            # x^T, token-tile-major: [p, t, c, tok]
            xT4 = persist.tile([128, NT, CT, 128], BF16, name="xT4", tag="xT4")
            # qkT[0..1] = Q^T head-pairs, qkT[2..3] = K^T head-pairs
            qkT = [persist.tile([128, N], BF16, name=f"qkT{f}", tag=f"qkT{f}")
                   for f in range(4)]
            # V with ones column per head: cols [65h .. 65h+64]
            vaug = [persist.tile([128, 65 * HPC], BF16, name=f"vaug{t}", tag=f"vaug{t}")
                    for t in range(NT)]
            # O^T stacked per head pair (rows 0-63 head 2p, 64-127 head 2p+1)
            oT2 = [persist.tile([128, N], BF16, name=f"oT2{p}", tag=f"oT2{p}")
                   for p in range(2)]
            wqk = [persist.tile([128, 2 * F], BF16, name=f"wqk{c}", tag=f"wqk{c}")
                   for c in range(CT)]
            wv = [persist.tile([128, F], BF16, name=f"wv{c}", tag=f"wv{c}")
                  for c in range(CT)]
            wp2 = [persist.tile([128, C], BF16, name=f"wp2{p}", tag=f"wp2{p}")
                   for p in range(2)]
            bqk_sb = [const.tile([128, 1], F32, name=f"bqk{f}", tag=f"bqk{f}")
                      for f in range(4)]
            bvb = const.tile([128, F], F32, name="bvb", tag="bvb")

            # preload the exp table set (one-time ~2.7us)
            scr = const.tile([1, 16], F32, name="scr", tag="scr")
            nc.scalar.activation(scr[:], ident[0:1, 0:16], AF.Exp)

            xload = ctx.enter_context(tc.tile_pool(name="xload", bufs=4))
            wstage = ctx.enter_context(tc.tile_pool(name="wstage", bufs=2))
            xbp = ctx.enter_context(tc.tile_pool(name="xbp", bufs=2))
            ptp = ctx.enter_context(tc.tile_pool(name="ptp", bufs=20))
            snorm = ctx.enter_context(tc.tile_pool(name="snorm", bufs=2))
            ysb = ctx.enter_context(tc.tile_pool(name="ysb", bufs=2))

            bpp = ctx.enter_context(
                tc.tile_pool(name="bpp", bufs=2, space=bass.MemorySpace.PSUM))
            cpp = ctx.enter_context(
                tc.tile_pool(name="cpp", bufs=2, space=bass.MemorySpace.PSUM))
            aux = ctx.enter_context(
                tc.tile_pool(name="aux", bufs=2, space=bass.MemorySpace.PSUM))

            # ---------------- DMA issue (front-loaded) ----------------
            # x rows (tp*256 + i*128 + p) -> xs[p, i*1024 + c]
            x_view = x_d.ap().rearrange("(tp i p) c -> tp p i c", tp=NT // 2, i=2)
            xs_tiles = []
            for tp in range(NT // 2):
                xs = xload.tile([128, 2048], F32, name="xs", tag="xs")
                xsv = xs.rearrange("p (i c) -> p i c", i=2)
                if tp < 2:
                    for i in range(2):
                        nc.sync.dma_start(xsv[:, i], x_view[tp][:, i])
                else:
                    nc.sync.dma_start(xsv, x_view[tp])
                xs_tiles.append(xs)

            # wqk + biases on the scalar HWDGE ring (fast, needed first)
            wqk_stage = []
            for c in range(CT):
                s = wstage.tile([128, 2 * F], F32, name="wqks", tag="wqks")
                nc.scalar.dma_start(s[:], wqk_d.ap()[c * 128:(c + 1) * 128, :])
                wqk_stage.append(s)
            for f in range(4):
                nc.scalar.dma_start(bqk_sb[f][:], bqk_d.ap()[f * 128:(f + 1) * 128, :])

            # wv / wp / bv on the gpsimd ring
            wv_stage = []
            for c in range(CT):
                s2 = wstage.tile([128, F], F32, name="wvs", tag="wvs")
                nc.gpsimd.dma_start(s2[:], wv_d.ap()[c * 128:(c + 1) * 128, :])
                wv_stage.append(s2)
            wp_stage = []
            for p in range(2):
                s3 = wstage.tile([128, C], F32, name="wps", tag="wps")
                nc.gpsimd.dma_start(s3[:], wp_d.ap()[p * 128:(p + 1) * 128, :])
                wp_stage.append(s3)
            bv1 = const.tile([1, F], F32, name="bv1", tag="bv1")
            nc.gpsimd.dma_start(bv1[:], bv_d.ap()[:])
            nc.gpsimd.partition_broadcast(bvb[:], bv1[:])

            # ---------------- helper emitters ----------------
            xb_tiles = {}

            def cast_tp(tp):
                xb = xbp.tile([128, 2048], BF16, name="xb", tag="xb")
                nc.vector.tensor_copy(xb[:], xs_tiles[tp][:])
                xb_tiles[tp] = xb

            def transpose_quad(t, cq):
                # 4 c-tiles of token-tile t -> xT4[:, t, 4cq:4cq+4, :]
                xb = xb_tiles[t // 2]
                tq = aux.tile([128, 512], BF16, name="tq", tag="aux")
                i = t % 2
                for j in range(4):
                    c = 4 * cq + j
                    nc.tensor.transpose(
                        tq[:, j * 128:(j + 1) * 128],
                        xb[:, i * 1024 + c * 128:i * 1024 + (c + 1) * 128],
                        ident[:])
                nc.vector.tensor_copy(
                    xT4[:, t, 4 * cq:4 * cq + 4, :],
                    tq.rearrange("p (c n) -> p c n", c=4))

            def qk_unit(f, ch):
                qp = aux.tile([128, 512], F32, name="qp", tag="aux")
                t0 = ch * 4
                for c in range(CT):
                    nc.tensor.matmul(
                        qp[:],
                        wqk[c][:, f * 128:(f + 1) * 128],
                        xT4[:, t0:t0 + 4, c, :],
                        start=(c == 0), stop=(c == CT - 1))
                nc.vector.tensor_scalar_add(
                    qkT[f][:, ch * 512:(ch + 1) * 512], qp[:], bqk_sb[f][:])

            def v_unit(t):
                vp = aux.tile([128, F], F32, name="vp", tag="aux")
                for c in range(CT):
                    nc.tensor.matmul(
                        vp[:], xT4[:, t, c, :], wv[c][:],
                        start=(c == 0), stop=(c == CT - 1))
                for h in range(HPC):
                    nc.vector.memset(vaug[t][:, 65 * h + 64:65 * h + 65], 1.0)
                vv = vaug[t].rearrange("p (h d) -> p h d", h=HPC)
                nc.vector.tensor_add(
                    vv[:, :, 0:D],
                    vp.rearrange("p (h d) -> p h d", h=HPC),
                    bvb.rearrange("p (h d) -> p h d", h=HPC))

            def yp_unit(t):
                for ch in range(2):
                    yp = aux.tile([128, 512], F32, name="yp", tag="aux")
                    for p in range(2):
                        nc.tensor.matmul(
                            yp[:],
                            oT2[p][:, t * 128:(t + 1) * 128],
                            wp2[p][:, ch * 512:(ch + 1) * 512],
                            start=(p == 0), stop=(p == 1))
                    ys = ysb.tile([128, 512], BF16, name="ys", tag="ys")
                    nc.vector.tensor_copy(ys[:], yp[:])
                    nc.sync.dma_start(
                        y_d.ap()[t * 128:(t + 1) * 128, ch * 512:(ch + 1) * 512],
                        ys[:])

            def cast_wqk(c):
                nc.vector.tensor_copy(wqk[c][:], wqk_stage[c][:])

            def cast_wv():
                for c in range(CT):
                    nc.vector.tensor_copy(wv[c][:], wv_stage[c][:])

            def cast_wp():
                for p in range(2):
                    nc.vector.tensor_copy(wp2[p][:], wp_stage[p][:])

            # ---------------- prefix ----------------
            # wqk casts + x casts/transposes for tiles 0-7, then the
            # first K/Q chunks so the exp stream can start
            for c in range(4):
                cast_wqk(c)
            cast_tp(0)
            for cq in range(2):
                transpose_quad(0, cq)
                transpose_quad(1, cq)
            for c in range(4, CT):
                cast_wqk(c)
            cast_tp(1)
            for cq in range(2):
                transpose_quad(2, cq)
                transpose_quad(3, cq)
            cast_tp(2)
            for cq in range(2):
                transpose_quad(4, cq)
                transpose_quad(5, cq)
            cast_tp(3)
            for cq in range(2):
                transpose_quad(6, cq)
                transpose_quad(7, cq)
            qk_unit(2, 0)   # K^T heads 0,1 keys 0-511
            qk_unit(0, 0)   # Q^T heads 0,1 queries 0-511
            cast_wv()

            # -------- filler queue (deadline + producer ordered) -----
            # every qk_unit/v_unit appears AFTER the cast/transpose
            # groups producing its xT4 tiles, and v_unit(mt) pops
            # before the attention loop emits PV(mt)
            fillers = []

            def defer(fn, *a):
                fillers.append(lambda: fn(*a))

            def cast_and_quads(tp):
                defer(cast_tp, tp)
                for i in range(2):
                    for cq in range(2):
                        defer(transpose_quad, 2 * tp + i, cq)

            defer(qk_unit, 2, 1)     # keys 512-1023 (tiles 4-7 in prefix)
            defer(v_unit, 0)
            defer(v_unit, 1)
            defer(v_unit, 2)
            cast_and_quads(4)        # tiles 8,9
            defer(v_unit, 3)
            defer(v_unit, 4)
            cast_and_quads(5)        # tiles 10,11
            defer(qk_unit, 2, 2)     # keys 1024-1535, by unit 8
            defer(v_unit, 5)
            defer(v_unit, 6)
            cast_and_quads(6)        # tiles 12,13
            defer(v_unit, 7)
            defer(v_unit, 8)
            cast_and_quads(7)        # tiles 14,15
            defer(qk_unit, 2, 3)     # keys 1536-2047, by unit 12
            defer(v_unit, 9)
            defer(v_unit, 10)
            defer(v_unit, 11)
            defer(qk_unit, 3, 0)     # pair-1 keys, by unit 16
            defer(qk_unit, 1, 0)     # pair-1 queries q0, by unit 16
            defer(v_unit, 12)
            defer(v_unit, 13)
            defer(v_unit, 14)
            defer(v_unit, 15)
            defer(qk_unit, 3, 1)     # by unit 20
            defer(qk_unit, 3, 2)     # by unit 24
            defer(qk_unit, 3, 3)     # by unit 28
            defer(qk_unit, 0, 1)     # quarter-1 queries, by unit 32
            defer(qk_unit, 1, 1)
            defer(cast_wp)
            defer(qk_unit, 0, 2)     # by unit 64
            defer(qk_unit, 1, 2)
            defer(qk_unit, 0, 3)     # by unit 96
            defer(qk_unit, 1, 3)

            # ---------------- fused attention ----------------
            FPW = 3   # filler pops per unit

            def emit_fillers(n):
                for _ in range(n):
                    if fillers:
                        fillers.pop(0)()

            def pv_pair(ptent, cp_h, cp_hp, pr):
                pt, mt = ptent
                h0 = 65 * (2 * pr)
                h1 = 65 * (2 * pr + 1)
                st, sp = (mt == 0), (mt == NT - 1)
                nc.tensor.matmul(
                    cp_h[:], vaug[mt][:, h0:h0 + 65], pt[:, 0],
                    start=st, stop=sp)
                nc.tensor.matmul(
                    cp_hp[:], vaug[mt][:, h1:h1 + 65], pt[:, 1],
                    start=st, stop=sp)

            def make_normalize(pr, qs, cp_h, cp_hp):
                def norm():
                    for rb, cp in ((0, cp_h), (64, cp_hp)):
                        s0 = snorm.tile([1, 512], F32, name="s0", tag="s0")
                        nc.vector.tensor_copy(s0[:], cp[64:65, :])
                        sr = snorm.tile([1, 512], F32, name="sr", tag="sr")
                        nc.vector.reciprocal_approx_fast(sr[:], s0[:])
                        sb = snorm.tile([64, 512], F32, name="sb", tag="sb")
                        nc.gpsimd.partition_broadcast(sb[:], sr[:])
                        nc.vector.tensor_mul(
                            oT2[pr][rb:rb + 64, qs:qs + 512], cp[0:64, :], sb[:])
                return norm

            for quarter in range(4):
                qs = quarter * 512
                for pr in range(2):
                    qt = qkT[pr]
                    kt = qkT[2 + pr]
                    cp_h = cpp.tile([65, 512], F32, name="cph", tag="cp")
                    cp_hp = cpp.tile([65, 512], F32, name="cphp", tag="cp")
                    pts = []
                    for mt in range(NT):
                        if len(pts) > PVLAG:
                            pv_pair(pts.pop(0), cp_h, cp_hp, pr)
                        emit_fillers(FPW)
                        bp = bpp.tile([128, 2, 512], F32, name="bp", tag="bp")
                        tp0 = dict(tile_position=(0, 0)) if PAIRED else {}
                        tp1 = dict(tile_position=(64, 0)) if PAIRED else {}
                        nc.tensor.matmul(
                            bp[:, 0], kt[0:64, mt * 128:(mt + 1) * 128],
                            qt[0:64, qs:qs + 512],
                            start=True, stop=True, **tp0)
                        nc.tensor.matmul(
                            bp[:, 1], kt[64:128, mt * 128:(mt + 1) * 128],
                            qt[64:128, qs:qs + 512],
                            start=True, stop=True, **tp1)
                        pt = ptp.tile([128, 2, 512], BF16, name="pt", tag="pt")
                        nc.scalar.activation(pt[:], bp[:], AF.Exp, scale=scale)
                        pts.append((pt, mt))
                    while pts:
                        pv_pair(pts.pop(0), cp_h, cp_hp, pr)
                    # normalize inline; PVLAG delays the next pair's first
                    # PV (the next reader of these cp slots) enough that
                    # this chain never blocks the PE
                    make_normalize(pr, qs, cp_h, cp_hp)()
                # output projection for the completed quarter (lag one)
                if quarter >= 1:
                    for t in range(4 * (quarter - 1), 4 * quarter):
                        defer(yp_unit, t)

            # tail
            while fillers:
                fillers.pop(0)()
            for t in range(12, 16):
                yp_unit(t)

    nc.compile()
    return nc


def _get_nc():
    if "nc" not in _CACHE:
        _CACHE["nc"] = _build()
    return _CACHE["nc"]


def _in_maps(q, W_qkv, b_qkv, W_proj):
    maps = []
    for core in range(NCORES):
        b, g = divmod(core, HPC)
        cols = slice(g * F, (g + 1) * F)
        maps.append({
            "x": q[b],
            "wqk": np.ascontiguousarray(
                np.concatenate([W_qkv[:, cols], W_qkv[:, C:][:, cols]], axis=1)),
            "wv": np.ascontiguousarray(W_qkv[:, 2 * C:][:, cols]),
            "wp": np.ascontiguousarray(W_proj[cols, :]),
            "bqk": np.ascontiguousarray(
                np.concatenate([b_qkv[cols], b_qkv[C:][cols]]).reshape(2 * F, 1)),
            "bv": np.ascontiguousarray(b_qkv[2 * C:][cols].reshape(1, F)),
        })
    return maps


def kernel(q, W_qkv, b_qkv, W_proj, b_proj):
    from concourse.bass_utils import run_bass_kernel_spmd

    q = np.ascontiguousarray(np.asarray(q, dtype=np.float32))
    W_qkv = np.ascontiguousarray(np.asarray(W_qkv, dtype=np.float32))
    b_qkv = np.ascontiguousarray(np.asarray(b_qkv, dtype=np.float32))
    W_proj = np.ascontiguousarray(np.asarray(W_proj, dtype=np.float32))
    b_proj = np.ascontiguousarray(np.asarray(b_proj, dtype=np.float32))

    nc = _get_nc()
    res = run_bass_kernel_spmd(nc, _in_maps(q, W_qkv, b_qkv, W_proj),
                               core_ids=list(range(NCORES)))

    out = np.zeros((B, N, C), dtype=np.float32)
    for core in range(NCORES):
        out[core // HPC] += np.asarray(res.results[core]["y"], dtype=np.float32)
    out += b_proj
    return out


# revision 20
# speedup vs baseline: 1.2426x; 1.0305x over previous
"""Trainium2 Bass kernel for nn_Attention_56470230008033.

Multi-head self-attention (B=2, N=2048, C=1024, H=16 heads, D=64),
k = v = q, full qkv projection + output projection.

Sharding over 8 NeuronCores: data parallel on batch (2) x tensor
parallel on heads (4 head-groups of 4 heads).

v3: single fused pipeline.  The PE is the bottleneck engine
(~200us: logits + PV are PSUM-drain-bound at 2x216ns per key-tile;
projections/transposes on top); the ScalarE exp stream (~171us) runs
underneath it.  Structure:
  - attention processed per (query-quarter 512, head-pair); unit =
    one key-tile: bp [128, 2, 512] f32 spans 2 PSUM banks (head h ->
    bank 2i, h' -> 2i+1), one exp [128,1024] covers both heads
  - PSUM: 2x bp (4 banks) + 2 cp [65,512] + 2 shared scratch = 8
  - denominators ride the PV matmul as a ones-column (M=65)
  - x load / transpose / qkv projections / output projection are
    emitted as fine-grained "fillers" inside the attention loop in
    deadline order; PV lags 3 units; normalize is deferred into the
    next pair so nothing blocks the exp stream
  - weights: wqk on the scalar HWDGE ring (fast path, needed first),
    x on sync ring, wv/wp/biases on the gpsimd ring
  - y written bf16 (host sums partials in f32)
"""

import sys

for _p in ("/opt/trn_rl_repo", "/opt/pypackages"):
    if _p not in sys.path:
        sys.path.append(_p)

import numpy as np

B, N, C, H = 2, 2048, 1024, 16
D = C // H            # 64 head dim
NCORES = 8
HPC = 4               # heads per core
F = HPC * D           # 256 features per core
NT = N // 128         # 16 token tiles
CT = C // 128         # 8 contraction tiles

PAIRED = True         # row-tiled logits pairs (tile_position)
PVLAG = 5             # PV units of lag behind exp

_CACHE = {}


def _build():
    from concourse import bacc, bass, mybir, tile, masks

    F32 = mybir.dt.float32
    BF16 = mybir.dt.bfloat16
    AF = mybir.ActivationFunctionType

    nc = bacc.Bacc(
        "TRN2",
        target_bir_lowering=False,
        debug=False,
        enable_asserts=False,
        num_devices=NCORES,
    )
    x_d = nc.dram_tensor("x", [N, C], F32, kind="ExternalInput")
    wqk_d = nc.dram_tensor("wqk", [C, 2 * F], F32, kind="ExternalInput")
    wv_d = nc.dram_tensor("wv", [C, F], F32, kind="ExternalInput")
    wp_d = nc.dram_tensor("wp", [F, C], F32, kind="ExternalInput")
    bqk_d = nc.dram_tensor("bqk", [2 * F, 1], F32, kind="ExternalInput")
    bv_d = nc.dram_tensor("bv", [1, F], F32, kind="ExternalInput")
    y_d = nc.dram_tensor("y", [N, C], BF16, kind="ExternalOutput")

    scale = float(D) ** -0.5

    with tile.TileContext(nc) as tc:
        from contextlib import ExitStack

        with ExitStack() as ctx:
            const = ctx.enter_context(tc.tile_pool(name="const", bufs=1))
            persist = ctx.enter_context(tc.tile_pool(name="persist", bufs=1))

            ident = const.tile([128, 128], BF16, name="ident", tag="ident")
            masks.make_identity(nc, ident[:])

            # x^T, token-tile-major: [p, t, c, tok]
            xT4 = persist.tile([128, NT, CT, 128], BF16, name="xT4", tag="xT4")
            # qkT[0..1] = Q^T head-pairs, qkT[2..3] = K^T head-pairs
            qkT = [persist.tile([128, N], BF16, name=f"qkT{f}", tag=f"qkT{f}")
                   for f in range(4)]
            # V with ones column per head: cols [65h .. 65h+64]
            vaug = [persist.tile([128, 65 * HPC], BF16, name=f"vaug{t}", tag=f"vaug{t}")
                    for t in range(NT)]
            # O^T stacked per head pair (rows 0-63 head 2p, 64-127 head 2p+1)
            oT2 = [persist.tile([128, N], BF16, name=f"oT2{p}", tag=f"oT2{p}")
                   for p in range(2)]
            wqk = [persist.tile([128, 2 * F], BF16, name=f"wqk{c}", tag=f"wqk{c}")
                   for c in range(CT)]
            wv = [persist.tile([128, F], BF16, name=f"wv{c}", tag=f"wv{c}")
                  for c in range(CT)]
            wp2 = [persist.tile([128, C], BF16, name=f"wp2{p}", tag=f"wp2{p}")
                   for p in range(2)]
            bqk_sb = [const.tile([128, 1], F32, name=f"bqk{f}", tag=f"bqk{f}")
                      for f in range(4)]
            bvb = const.tile([128, F], F32, name="bvb", tag="bvb")

            # preload the exp table set (one-time ~2.7us)
            scr = const.tile([1, 16], F32, name="scr", tag="scr")
            nc.scalar.activation(scr[:], ident[0:1, 0:16], AF.Exp)

            xload = ctx.enter_context(tc.tile_pool(name="xload", bufs=4))
            wstage = ctx.enter_context(tc.tile_pool(name="wstage", bufs=8))
            wstage2 = ctx.enter_context(tc.tile_pool(name="wstage2", bufs=2))
            xbp = ctx.enter_context(tc.tile_pool(name="xbp", bufs=2))
            ptp = ctx.enter_context(tc.tile_pool(name="ptp", bufs=18))
            snorm = ctx.enter_context(tc.tile_pool(name="snorm", bufs=2))
            ysb = ctx.enter_context(tc.tile_pool(name="ysb", bufs=2))

            bpp = ctx.enter_context(
                tc.tile_pool(name="bpp", bufs=2, space=bass.MemorySpace.PSUM))
            cpp = ctx.enter_context(
                tc.tile_pool(name="cpp", bufs=2, space=bass.MemorySpace.PSUM))
            aux = ctx.enter_context(
                tc.tile_pool(name="aux", bufs=2, space=bass.MemorySpace.PSUM))

            # ---------------- DMA issue (front-loaded) ----------------
            # x rows (tp*256 + i*128 + p) -> xs[p, i*1024 + c]
            x_view = x_d.ap().rearrange("(tp i p) c -> tp p i c", tp=NT // 2, i=2)
            xs_tiles = []
            for tp in range(NT // 2):
                xs = xload.tile([128, 2048], F32, name="xs", tag="xs")
                xsv = xs.rearrange("p (i c) -> p i c", i=2)
                if tp < 2:
                    for i in range(2):
                        nc.sync.dma_start(xsv[:, i], x_view[tp][:, i])
                else:
                    nc.sync.dma_start(xsv, x_view[tp])
                xs_tiles.append(xs)

            # wqk + biases on the scalar HWDGE ring (fast, needed first)
            wqk_stage = []
            for c in range(CT):
                s = wstage.tile([128, 2 * F], F32, name="wqks", tag="wqks")
                nc.scalar.dma_start(s[:], wqk_d.ap()[c * 128:(c + 1) * 128, :])
                wqk_stage.append(s)
            for f in range(4):
                nc.scalar.dma_start(bqk_sb[f][:], bqk_d.ap()[f * 128:(f + 1) * 128, :])

            # wv / wp / bv on the gpsimd ring
            wv_stage = []
            for c in range(CT):
                s2 = wstage.tile([128, F], F32, name="wvs", tag="wvs")
                nc.gpsimd.dma_start(s2[:], wv_d.ap()[c * 128:(c + 1) * 128, :])
                wv_stage.append(s2)
            wp_stage = []
            for p in range(2):
                s3 = wstage2.tile([128, C], F32, name="wps", tag="wps")
                nc.gpsimd.dma_start(s3[:], wp_d.ap()[p * 128:(p + 1) * 128, :])
                wp_stage.append(s3)
            bv1 = const.tile([1, F], F32, name="bv1", tag="bv1")
            nc.gpsimd.dma_start(bv1[:], bv_d.ap()[:])
            nc.gpsimd.partition_broadcast(bvb[:], bv1[:])

            # ---------------- helper emitters ----------------
            xb_tiles = {}

            def cast_tp(tp):
                xb = xbp.tile([128, 2048], BF16, name="xb", tag="xb")
                nc.vector.tensor_copy(xb[:], xs_tiles[tp][:])
                xb_tiles[tp] = xb

            def transpose_quad(t, cq):
                # 4 c-tiles of token-tile t -> xT4[:, t, 4cq:4cq+4, :]
                xb = xb_tiles[t // 2]
                tq = aux.tile([128, 512], BF16, name="tq", tag="aux")
                i = t % 2
                for j in range(4):
                    c = 4 * cq + j
                    nc.tensor.transpose(
                        tq[:, j * 128:(j + 1) * 128],
                        xb[:, i * 1024 + c * 128:i * 1024 + (c + 1) * 128],
                        ident[:])
                nc.vector.tensor_copy(
                    xT4[:, t, 4 * cq:4 * cq + 4, :],
                    tq.rearrange("p (c n) -> p c n", c=4))

            def qk_unit(f, ch):
                qp = aux.tile([128, 512], F32, name="qp", tag="aux")
                t0 = ch * 4
                for c in range(CT):
                    nc.tensor.matmul(
                        qp[:],
                        wqk[c][:, f * 128:(f + 1) * 128],
                        xT4[:, t0:t0 + 4, c, :],
                        start=(c == 0), stop=(c == CT - 1))
                nc.vector.tensor_scalar_add(
                    qkT[f][:, ch * 512:(ch + 1) * 512], qp[:], bqk_sb[f][:])

            def v_unit(t):
                vp = aux.tile([128, F], F32, name="vp", tag="aux")
                for c in range(CT):
                    nc.tensor.matmul(
                        vp[:], xT4[:, t, c, :], wv[c][:],
                        start=(c == 0), stop=(c == CT - 1))
                for h in range(HPC):
                    nc.vector.memset(vaug[t][:, 65 * h + 64:65 * h + 65], 1.0)
                vv = vaug[t].rearrange("p (h d) -> p h d", h=HPC)
                nc.vector.tensor_add(
                    vv[:, :, 0:D],
                    vp.rearrange("p (h d) -> p h d", h=HPC),
                    bvb.rearrange("p (h d) -> p h d", h=HPC))

            def yp_unit(t):
                for ch in range(2):
                    yp = aux.tile([128, 512], F32, name="yp", tag="aux")
                    for p in range(2):
                        nc.tensor.matmul(
                            yp[:],
                            oT2[p][:, t * 128:(t + 1) * 128],
                            wp2[p][:, ch * 512:(ch + 1) * 512],
                            start=(p == 0), stop=(p == 1))
                    ys = ysb.tile([128, 512], BF16, name="ys", tag="ys")
                    nc.vector.tensor_copy(ys[:], yp[:])
                    nc.sync.dma_start(
                        y_d.ap()[t * 128:(t + 1) * 128, ch * 512:(ch + 1) * 512],
                        ys[:])

            def cast_wqk(c):
                nc.vector.tensor_copy(wqk[c][:], wqk_stage[c][:])

            def cast_wv():
                for c in range(CT):
                    nc.vector.tensor_copy(wv[c][:], wv_stage[c][:])

            def cast_wp():
                for p in range(2):
                    nc.vector.tensor_copy(wp2[p][:], wp_stage[p][:])

            # ---------------- prefix ----------------
            # x casts/transposes for tiles 0-3 and wqk casts, then the
            # first K/Q chunks so the exp stream can start
            cast_tp(0)
            for cq in range(2):
                transpose_quad(0, cq)
                transpose_quad(1, cq)
            for c in range(CT):
                cast_wqk(c)
            cast_tp(1)
            for cq in range(2):
                transpose_quad(2, cq)
                transpose_quad(3, cq)
            qk_unit(2, 0)   # K^T heads 0,1 keys 0-511
            qk_unit(0, 0)   # Q^T heads 0,1 queries 0-511

            # -------- filler queue (deadline + producer ordered) -----
            # every qk_unit/v_unit appears AFTER the cast/transpose
            # groups producing its xT4 tiles, and v_unit(mt) pops
            # before the attention loop emits PV(mt)
            fillers = []

            def defer(fn, *a):
                fillers.append(lambda: fn(*a))

            def cast_and_quads(tp):
                defer(cast_tp, tp)
                for i in range(2):
                    for cq in range(2):
                        defer(transpose_quad, 2 * tp + i, cq)

            defer(cast_wv)
            cast_and_quads(2)        # tiles 4,5
            cast_and_quads(3)        # tiles 6,7
            defer(qk_unit, 2, 1)     # keys 512-1023, by unit 4
            defer(v_unit, 0)
            defer(v_unit, 1)
            cast_and_quads(4)        # tiles 8,9
            defer(v_unit, 2)
            defer(v_unit, 3)
            cast_and_quads(5)        # tiles 10,11
            defer(qk_unit, 2, 2)     # keys 1024-1535, by unit 8
            defer(v_unit, 4)
            defer(v_unit, 5)
            cast_and_quads(6)        # tiles 12,13
            defer(v_unit, 6)
            defer(v_unit, 7)
            cast_and_quads(7)        # tiles 14,15
            defer(qk_unit, 2, 3)     # keys 1536-2047, by unit 12
            defer(v_unit, 8)
            defer(v_unit, 9)
            defer(v_unit, 10)
            defer(v_unit, 11)
            defer(qk_unit, 3, 0)     # pair-1 keys, by unit 16
            defer(qk_unit, 1, 0)     # pair-1 queries q0, by unit 16
            defer(v_unit, 12)
            defer(v_unit, 13)
            defer(v_unit, 14)
            defer(v_unit, 15)
            defer(qk_unit, 3, 1)     # by unit 20
            defer(qk_unit, 3, 2)     # by unit 24
            defer(qk_unit, 3, 3)     # by unit 28
            defer(qk_unit, 0, 1)     # quarter-1 queries, by unit 32
            defer(qk_unit, 1, 1)
            defer(cast_wp)
            defer(qk_unit, 0, 2)     # by unit 64
            defer(qk_unit, 1, 2)
            defer(qk_unit, 0, 3)     # by unit 96
            defer(qk_unit, 1, 3)

            # ---------------- fused attention ----------------
            unit_ctr = [0]

            def emit_fillers():
                n = 4 if unit_ctr[0] < 16 else 2
                for _ in range(n):
                    if fillers:
                        fillers.pop(0)()
                unit_ctr[0] += 1

            def pv_pair(ptent, cp_h, cp_hp, pr):
                pt, mt = ptent
                h0 = 65 * (2 * pr)
                h1 = 65 * (2 * pr + 1)
                st, sp = (mt == 0), (mt == NT - 1)
                nc.tensor.matmul(
                    cp_h[:], vaug[mt][:, h0:h0 + 65], pt[:, 0],
                    start=st, stop=sp)
                nc.tensor.matmul(
                    cp_hp[:], vaug[mt][:, h1:h1 + 65], pt[:, 1],
                    start=st, stop=sp)

            def make_normalize(pr, qs, cp_h, cp_hp):
                def norm():
                    for rb, cp in ((0, cp_h), (64, cp_hp)):
                        s0 = snorm.tile([1, 512], F32, name="s0", tag="s0")
                        nc.vector.tensor_copy(s0[:], cp[64:65, :])
                        sr = snorm.tile([1, 512], F32, name="sr", tag="sr")
                        nc.vector.reciprocal_approx_fast(sr[:], s0[:])
                        sb = snorm.tile([64, 512], F32, name="sb", tag="sb")
                        nc.gpsimd.partition_broadcast(sb[:], sr[:])
                        nc.vector.tensor_mul(
                            oT2[pr][rb:rb + 64, qs:qs + 512], cp[0:64, :], sb[:])
                return norm

            for quarter in range(4):
                qs = quarter * 512
                for pr in range(2):
                    qt = qkT[pr]
                    kt = qkT[2 + pr]
                    cp_h = cpp.tile([65, 512], F32, name="cph", tag="cp")
                    cp_hp = cpp.tile([65, 512], F32, name="cphp", tag="cp")
                    pts = []
                    for mt in range(NT):
                        emit_fillers()
                        if len(pts) > PVLAG:
                            pv_pair(pts.pop(0), cp_h, cp_hp, pr)
                        bp = bpp.tile([128, 2, 512], F32, name="bp", tag="bp")
                        tp0 = dict(tile_position=(0, 0)) if PAIRED else {}
                        tp1 = dict(tile_position=(64, 0)) if PAIRED else {}
                        nc.tensor.matmul(
                            bp[:, 0], kt[0:64, mt * 128:(mt + 1) * 128],
                            qt[0:64, qs:qs + 512],
                            start=True, stop=True, **tp0)
                        nc.tensor.matmul(
                            bp[:, 1], kt[64:128, mt * 128:(mt + 1) * 128],
                            qt[64:128, qs:qs + 512],
                            start=True, stop=True, **tp1)
                        pt = ptp.tile([128, 2, 512], BF16, name="pt", tag="pt")
                        nc.scalar.activation(pt[:], bp[:], AF.Exp, scale=scale)
                        pts.append((pt, mt))
                    while pts:
                        pv_pair(pts.pop(0), cp_h, cp_hp, pr)
                    # normalize inline; PVLAG delays the next pair's first
                    # PV (the next reader of these cp slots) enough that
                    # this chain never blocks the PE
                    make_normalize(pr, qs, cp_h, cp_hp)()
                # output projection for the completed quarter (lag one)
                if quarter >= 1:
                    for t in range(4 * (quarter - 1), 4 * quarter):
                        defer(yp_unit, t)

            # tail
            while fillers:
                fillers.pop(0)()
            for t in range(12, 16):
                yp_unit(t)

    nc.compile()
    return nc


def _get_nc():
    if "nc" not in _CACHE:
        _CACHE["nc"] = _build()
    return _CACHE["nc"]


def _in_maps(q, W_qkv, b_qkv, W_proj):
    maps = []
    for core in range(NCORES):
        b, g = divmod(core, HPC)
        cols = slice(g * F, (g + 1) * F)
        maps.append({
            "x": q[b],
            "wqk": np.ascontiguousarray(
                np.concatenate([W_qkv[:, cols], W_qkv[:, C:][:, cols]], axis=1)),
            "wv": np.ascontiguousarray(W_qkv[:, 2 * C:][:, cols]),
            "wp": np.ascontiguousarray(W_proj[cols, :]),
            "bqk": np.ascontiguousarray(
                np.concatenate([b_qkv[cols], b_qkv[C:][cols]]).reshape(2 * F, 1)),
            "bv": np.ascontiguousarray(b_qkv[2 * C:][cols].reshape(1, F)),
        })
    return maps


def kernel(q, W_qkv, b_qkv, W_proj, b_proj):
    from concourse.bass_utils import run_bass_kernel_spmd

    q = np.ascontiguousarray(np.asarray(q, dtype=np.float32))
    W_qkv = np.ascontiguousarray(np.asarray(W_qkv, dtype=np.float32))
    b_qkv = np.ascontiguousarray(np.asarray(b_qkv, dtype=np.float32))
    W_proj = np.ascontiguousarray(np.asarray(W_proj, dtype=np.float32))
    b_proj = np.ascontiguousarray(np.asarray(b_proj, dtype=np.float32))

    nc = _get_nc()
    res = run_bass_kernel_spmd(nc, _in_maps(q, W_qkv, b_qkv, W_proj),
                               core_ids=list(range(NCORES)))

    out = np.zeros((B, N, C), dtype=np.float32)
    for core in range(NCORES):
        out[core // HPC] += np.asarray(res.results[core]["y"], dtype=np.float32)
    out += b_proj
    return out
